# revision 1
# baseline (speedup 1.0000x reference)
"""CVRP decoder kernel for Trainium2 (8 NeuronCores, batch-data-parallel).

Computes, per batch b (B=64, P=64, N=1000, H=128):
    q_graph   = mean_n(emb) @ Wq_graph
    q_first   = encoded_q1 @ Wq_first
    q_last    = emb[last_node] @ Wq_last
    q_visited = (vis01 @ emb / N) @ W_visited          (vis01 = isneginf(mask))
    final_q   = sum of the above + load*W_load + b_load
    score     = final_q @ emb^T / sqrt(H) - dists[last_node] / sqrt(2)
    probs     = softmax(10*tanh(score) + (-BIG if visited))

Sharding: batch dim across the 8 cores (pure data parallel), 8 batches per
core processed as 4 pairs of 2 batches stacked on the 128 SBUF partitions.
"""

import json
import math
import numpy as np
from contextlib import ExitStack

import concourse.bass as bass
import concourse.mybir as mybir
import concourse.tile as tile
from concourse.bass_utils import run_bass_kernel_spmd
from concourse.masks import make_identity


def _split_excess_waits(bir_bytes: bytes, max_waits: int = 1) -> bytes:
    """Walrus in this image rejects instructions carrying too many sem waits
    ("Too many sync wait commands", e.g. on Tile's kernel-tail Drain).
    Hoist excess waits onto preceding same-engine EventSemaphore carriers
    (pure sync ops) — sems are monotonic, so a chain of instructions whose
    waits partition the original list is equivalent."""
    d = json.loads(bir_bytes)
    n = [0]
    for fn in d.get("functions", []):
        for blk in fn.get("blocks", []):
            out = []
            for ins in blk.get("instructions", []):
                si = ins.get("sync_info") or {}
                waits = si.get("on_wait") or []
                if len(waits) > max_waits:
                    extra, keep = waits[:-max_waits], waits[-max_waits:]
                    ins["sync_info"]["on_wait"] = keep
                    for i in range(0, len(extra), max_waits):
                        n[0] += 1
                        carrier = {
                            "name": f"I-waitsplit-{n[0]}",
                            "opcode": "EventSemaphore",
                            "engine": ins["engine"],
                            "ins": [],
                            "outs": [],
                            "sync_info": {
                                "on_update": [],
                                "on_wait": extra[i:i + max_waits],
                            },
                        }
                        if "debug" in ins:
                            carrier["debug"] = ins["debug"]
                        out.append(carrier)
                out.append(ins)
            blk["instructions"] = out
    return json.dumps(d).encode()


def _install_walrus_shim():
    import concourse.bass2jax as b2j
    import concourse.bass_utils as bu
    if getattr(bu, "_waitsplit_installed", False):
        return
    real = bu.compile_bir_kernel

    def patched(bir_json, tmpdir, neff_name="file.neff", **kw):
        if isinstance(bir_json, (bytes, bytearray, str)):
            if isinstance(bir_json, str):
                bir_json = bir_json.encode()
            bir_json = _split_excess_waits(bir_json)
        return real(bir_json, tmpdir, neff_name=neff_name, **kw)

    bu.compile_bir_kernel = patched
    b2j.compile_bir_kernel = patched
    bu._waitsplit_installed = True


_install_walrus_shim()

F32 = mybir.dt.float32
I32 = mybir.dt.int32
OP = mybir.AluOpType
AF = mybir.ActivationFunctionType

B, P, N, H = 64, 64, 1000, 128
NCORES = 8
NB = B // NCORES          # 8 batches per core
NPAIR = NB // 2           # 4 pairs
NCHUNK = 8                # n-chunks of <=128 rows: 7*128 + 104
CHUNK_CNT = [128] * 7 + [N - 7 * 128]   # [128]*7 + [104]

MASK_NEG = -1000.0        # additive bias for visited nodes (pre x10 exp scale)
QV_SCALE = -1.0 / (1000.0 * N)   # undo MASK_NEG and the /N in one eviction
FQ_SCALE = math.sqrt(2.0) / math.sqrt(H)   # = 0.125 exactly
TANH_SCALE = 1.0 / math.sqrt(2.0)
TANH_CLIP = 10.0


def build_nc():
    nc = bass.Bass()

    dists = nc.dram_tensor("dists", [NB * N, N], F32, kind="ExternalInput")
    emb = nc.dram_tensor("emb", [NB * N, H], F32, kind="ExternalInput")
    eq1 = nc.dram_tensor("eq1", [NB * P, H], F32, kind="ExternalInput")
    lastnode = nc.dram_tensor("lastnode", [NB * P, 1], I32, kind="ExternalInput")
    loadv = nc.dram_tensor("loadv", [NPAIR, 128], F32, kind="ExternalInput")
    maskt = nc.dram_tensor("maskt", [NB * P, N], F32, kind="ExternalInput")
    wq_graph = nc.dram_tensor("wq_graph", [H, H], F32, kind="ExternalInput")
    wq_first = nc.dram_tensor("wq_first", [H, H], F32, kind="ExternalInput")
    wq_last = nc.dram_tensor("wq_last", [H, H], F32, kind="ExternalInput")
    w_visited = nc.dram_tensor("w_visited", [H, H], F32, kind="ExternalInput")
    w_load = nc.dram_tensor("w_load", [1, H], F32, kind="ExternalInput")
    b_load = nc.dram_tensor("b_load", [1, H], F32, kind="ExternalInput")
    probs = nc.dram_tensor("probs", [NB * P, N], F32, kind="ExternalOutput")

    with tile.TileContext(nc) as tc:
        with ExitStack() as ctx:
            const = ctx.enter_context(tc.tile_pool(name="const", bufs=1))
            sb = ctx.enter_context(tc.tile_pool(name="sb", bufs=3))
            sbe = ctx.enter_context(tc.tile_pool(name="sbe", bufs=4))
            ps_big = ctx.enter_context(
                tc.tile_pool(name="ps_big", bufs=4, space="PSUM"))
            ps_mb = ctx.enter_context(
                tc.tile_pool(name="ps_mb", bufs=2, space="PSUM"))
            ps_small = ctx.enter_context(
                tc.tile_pool(name="ps_small", bufs=2, space="PSUM"))

            # ---- constants ----
            ident = const.tile([128, 128], F32, tag="ident")
            make_identity(nc, ident[:])
            ones_row = const.tile([1, 128], F32, tag="ones_row")
            nc.gpsimd.memset(ones_row[:], 1.0)

            wg = const.tile([H, H], F32, tag="wg")
            nc.sync.dma_start(wg[:], wq_graph[:])
            wf = const.tile([H, H], F32, tag="wf")
            nc.sync.dma_start(wf[:], wq_first[:])
            wl = const.tile([H, H], F32, tag="wl")
            nc.sync.dma_start(wl[:], wq_last[:])
            wv = const.tile([H, H], F32, tag="wv")
            nc.sync.dma_start(wv[:], w_visited[:])
            wld = const.tile([1, H], F32, tag="wld")
            nc.sync.dma_start(wld[:], w_load[:])
            bld = const.tile([1, H], F32, tag="bld")
            nc.sync.dma_start(bld[:], b_load[:])

            for pr in range(NPAIR):
                b0 = 2 * pr              # first batch of the pair (core-local)
                r0 = 128 * pr            # row offset into [NB*P, ...] tensors

                # ---- indices: flat row index into [NB*N, ...] = idx + 1000*b
                idxr = sb.tile([128, 1], I32, tag="idxr")
                nc.sync.dma_start(idxr[:], lastnode[r0:r0 + 128, :])
                adj = sb.tile([128, 1], I32, tag="adj")
                nc.gpsimd.memset(adj[0:64, :], N * b0)
                nc.gpsimd.memset(adj[64:128, :], N * (b0 + 1))
                idxa = sb.tile([128, 1], I32, tag="idxa")
                nc.vector.tensor_tensor(out=idxa[:], in0=idxr[:], in1=adj[:],
                                        op=OP.add)

                # ---- gathers: dist rows + last-node embedding rows
                distg = sb.tile([128, N], F32, tag="distg")
                nc.gpsimd.indirect_dma_start(
                    out=distg[:], out_offset=None, in_=dists[:],
                    in_offset=bass.IndirectOffsetOnAxis(ap=idxa[:, 0:1], axis=0))
                lastemb = sb.tile([128, H], F32, tag="lastemb")
                nc.gpsimd.indirect_dma_start(
                    out=lastemb[:], out_offset=None, in_=emb[:],
                    in_offset=bass.IndirectOffsetOnAxis(ap=idxa[:, 0:1], axis=0))

                # ---- plain loads
                mk = sb.tile([128, N], F32, tag="mk")
                nc.sync.dma_start(mk[:], maskt[r0:r0 + 128, :])
                eq1s = sb.tile([128, H], F32, tag="eq1s")
                nc.sync.dma_start(eq1s[:], eq1[r0:r0 + 128, :])
                ldrow = sb.tile([1, 128], F32, tag="ldrow")
                nc.sync.dma_start(ldrow[:], loadv[pr:pr + 1, :])

                emb_n = []
                for j in range(2):
                    e = sbe.tile([128, NCHUNK, H], F32, tag="embn")
                    base = (b0 + j) * N
                    nc.sync.dma_start(e[:, 0:7, :],
                                      emb[base:base + 896, :]
                                      .rearrange("(c p) h -> p c h", p=128))
                    nc.sync.dma_start(e[0:104, 7, :], emb[base + 896:base + N, :])
                    emb_n.append(e)

                # ---- maskbias = (mask < -1e30) * (-1000)   {0, -1000}
                mb = sb.tile([128, N], F32, tag="mb")
                nc.vector.tensor_scalar(out=mb[:], in0=mk[:],
                                        scalar1=-1e30, scalar2=MASK_NEG,
                                        op0=OP.is_lt, op1=OP.mult)

                # ---- transpose maskbias -> mbT [n, 2p] chunks (PE, packed psum)
                mbT = sb.tile([128, NCHUNK, 128], F32, tag="mbT")
                for g in range(2):
                    pmb = ps_mb.tile([128, 4, 128], F32, tag="pmb")
                    for j in range(4):
                        c = 4 * g + j
                        cnt = CHUNK_CNT[c]
                        nc.tensor.transpose(
                            out=pmb[0:cnt, j, :],
                            in_=mb[:, 128 * c:128 * c + cnt],
                            identity=ident[:])
                    if g == 0:
                        nc.scalar.copy(mbT[:, 0:4, :], pmb[:])
                    else:
                        nc.scalar.copy(mbT[:, 4:7, :], pmb[:, 0:3, :])
                        nc.scalar.copy(mbT[0:104, 7, :], pmb[0:104, 3, :])

                # ---- embT per batch: [h, n] via PE transposes; mean via accum
                embT = []
                macc = []
                for j in range(2):
                    et = sbe.tile([128, 1024], F32, tag="embT")
                    acc = sb.tile([128, 2], F32, tag="macc")
                    psA = ps_big.tile([128, 512], F32, tag="psbig")
                    for c in range(4):
                        nc.tensor.transpose(
                            out=psA[:, 128 * c:128 * (c + 1)],
                            in_=emb_n[j][:, c, :],
                            identity=ident[:])
                    nc.scalar.activation(et[:, 0:512], psA[:], AF.Copy,
                                         accum_out=acc[:, 0:1])
                    psB = ps_big.tile([128, 488], F32, tag="psbig")
                    for c in range(4, NCHUNK):
                        cnt = CHUNK_CNT[c]
                        nc.tensor.transpose(
                            out=psB[:, 128 * c - 512:128 * c - 512 + cnt],
                            in_=emb_n[j][0:cnt, c, :],
                            identity=ident[0:cnt, 0:cnt])
                    nc.scalar.activation(et[:, 512:1000], psB[:], AF.Copy,
                                         accum_out=acc[:, 1:2])
                    embT.append(et)
                    macc.append(acc)

                # mean broadcast over the p dim: [128, 128]
                meanrep = sb.tile([128, 128], F32, tag="meanrep")
                for j in range(2):
                    ms = sb.tile([128, 1], F32, tag="ms")
                    nc.vector.tensor_tensor(out=ms[:], in0=macc[j][:, 0:1],
                                            in1=macc[j][:, 1:2], op=OP.add)
                    nc.vector.tensor_scalar(
                        out=meanrep[:, 64 * j:64 * j + 64],
                        in0=ms[:, 0:1].to_broadcast([128, 64]),
                        scalar1=1.0 / N, scalar2=None, op0=OP.mult)

                # ---- input transposes (whole pair at once)
                ps_t = ps_small.tile([128, 128], F32, tag="pss")
                nc.tensor.transpose(out=ps_t[:], in_=eq1s[:], identity=ident[:])
                eq1T = sb.tile([128, 128], F32, tag="eq1T")
                nc.vector.tensor_copy(out=eq1T[:], in_=ps_t[:])

                ps_t2 = ps_small.tile([128, 128], F32, tag="pss")
                nc.tensor.transpose(out=ps_t2[:], in_=lastemb[:],
                                    identity=ident[:])
                lastembT = sb.tile([128, 128], F32, tag="lastembT")
                nc.vector.tensor_copy(out=lastembT[:], in_=ps_t2[:])

                # ---- q_visited pre: psum[h, p] per batch
                qvs = sb.tile([128, 2, 64], F32, tag="qvs")
                for j in range(2):
                    pqv = ps_small.tile([128, 64], F32, tag="pss")
                    for c in range(NCHUNK):
                        cnt = CHUNK_CNT[c]
                        nc.tensor.matmul(
                            pqv[:],
                            lhsT=emb_n[j][0:cnt, c, :],
                            rhs=mbT[0:cnt, c, 64 * j:64 * j + 64],
                            start=(c == 0), stop=(c == NCHUNK - 1))
                    # psum = -1000 * sum_vis emb ; rescale to qv_pre/N
                    nc.vector.tensor_scalar(out=qvs[:, j, :], in0=pqv[:],
                                            scalar1=QV_SCALE, scalar2=None,
                                            op0=OP.mult)

                # ---- final_q^T accumulation: psum [h, 2p]
                pfq = ps_small.tile([128, 128], F32, tag="pss")
                nc.tensor.matmul(pfq[:], lhsT=wf[:], rhs=eq1T[:],
                                 start=True, stop=False)
                nc.tensor.matmul(pfq[:], lhsT=wl[:], rhs=lastembT[:],
                                 start=False, stop=False)
                nc.tensor.matmul(pfq[:], lhsT=wg[:], rhs=meanrep[:],
                                 start=False, stop=False)
                nc.tensor.matmul(pfq[:], lhsT=wv[:], rhs=qvs[:],
                                 start=False, stop=False)
                nc.tensor.matmul(pfq[:], lhsT=wld[:], rhs=ldrow[:],
                                 start=False, stop=False)
                nc.tensor.matmul(pfq[:], lhsT=bld[:], rhs=ones_row[:],
                                 start=False, stop=True)
                fqT = sb.tile([128, 128], F32, tag="fqT")
                nc.scalar.mul(fqT[:], pfq[:], FQ_SCALE)

                # ---- score matmuls + bias + tanh + mask + softmax
                u = sb.tile([128, N], F32, tag="u")
                for (n0, n1) in ((0, 512), (512, N)):
                    psc = ps_big.tile([128, n1 - n0], F32, tag="psbig")
                    for j in range(2):
                        nc.tensor.matmul(
                            psc[64 * j:64 * j + 64, :],
                            lhsT=fqT[:, 64 * j:64 * j + 64],
                            rhs=embT[j][:, n0:n1],
                            start=True, stop=True)
                    nc.vector.scalar_tensor_tensor(
                        out=u[:, n0:n1], in0=psc[:], scalar=0.0,
                        in1=distg[:, n0:n1],
                        op0=OP.bypass, op1=OP.subtract)

                t = sb.tile([128, N], F32, tag="t")
                nc.scalar.activation(t[:], u[:], AF.Tanh, scale=TANH_SCALE)
                w = sb.tile([128, N], F32, tag="w")
                nc.vector.tensor_tensor(out=w[:], in0=t[:], in1=mb[:], op=OP.add)

                e = sb.tile([128, N], F32, tag="e")
                ssum = sb.tile([128, 1], F32, tag="ssum")
                nc.scalar.activation(e[:], w[:], AF.Exp, scale=TANH_CLIP,
                                     accum_out=ssum[:])
                rec = sb.tile([128, 1], F32, tag="rec")
                nc.vector.reciprocal(out=rec[:], in_=ssum[:])
                pout = sb.tile([128, N], F32, tag="pout")
                nc.scalar.activation(pout[:], e[:], AF.Copy,
                                     scale=rec[:, 0:1])
                nc.sync.dma_start(probs[r0:r0 + 128, :], pout[:])

    return nc


_CACHE = {}


def _get_nc():
    if "nc" not in _CACHE:
        _CACHE["nc"] = build_nc()
    return _CACHE["nc"]


def _shard_inputs(inputs):
    dists = np.ascontiguousarray(inputs["dists"], dtype=np.float32)
    embeddings = np.ascontiguousarray(inputs["embeddings"], dtype=np.float32)
    encoded_q1 = np.ascontiguousarray(inputs["encoded_q1"], dtype=np.float32)
    last_node = np.ascontiguousarray(inputs["last_node"]).astype(np.int32)
    load = np.ascontiguousarray(inputs["load"], dtype=np.float32)
    mask = np.ascontiguousarray(inputs["group_ninf_mask"], dtype=np.float32)
    # -inf -> large finite negative: identical kernel behavior (the visited
    # test is `< -1e30`), but keeps every downstream ALU input finite.
    mask = np.maximum(mask, np.float32(-3e38))
    in_maps = []
    for c in range(NCORES):
        s = slice(c * NB, (c + 1) * NB)
        in_maps.append(dict(
            dists=dists[s].reshape(NB * N, N),
            emb=embeddings[s].reshape(NB * N, H),
            eq1=encoded_q1[s].reshape(NB * P, H),
            lastnode=last_node[s].reshape(NB * P, 1),
            loadv=load[s].reshape(NPAIR, 128),
            maskt=mask[s].reshape(NB * P, N),
            wq_graph=np.ascontiguousarray(inputs["Wq_graph"], dtype=np.float32),
            wq_first=np.ascontiguousarray(inputs["Wq_first"], dtype=np.float32),
            wq_last=np.ascontiguousarray(inputs["Wq_last"], dtype=np.float32),
            w_visited=np.ascontiguousarray(inputs["W_visited"], dtype=np.float32),
            w_load=np.ascontiguousarray(inputs["W_load"], dtype=np.float32)
                .reshape(1, H),
            b_load=np.ascontiguousarray(inputs["b_load"], dtype=np.float32)
                .reshape(1, H),
        ))
    return in_maps


def _run(inputs, trace=False, **kw):
    nc = _get_nc()
    in_maps = _shard_inputs(inputs)
    res = run_bass_kernel_spmd(nc, in_maps, list(range(NCORES)),
                               trace=trace, **kw)
    out = np.concatenate(
        [r["probs"].reshape(NB, P, N) for r in res.results], axis=0)
    return out, res


def kernel(**inputs) -> np.ndarray:
    out, _ = _run(inputs)
    return out



# revision 5
# speedup vs baseline: 1.7447x; 1.7447x over previous
"""CVRP decoder kernel for Trainium2 (8 NeuronCores, batch-data-parallel).

Computes, per batch b (B=64, P=64, N=1000, H=128):
    q_graph   = mean_n(emb) @ Wq_graph
    q_first   = encoded_q1 @ Wq_first
    q_last    = emb[last_node] @ Wq_last
    q_visited = (vis01 @ emb / N) @ W_visited          (vis01 = isneginf(mask))
    final_q   = sum of the above + load*W_load + b_load
    score     = final_q @ emb^T / sqrt(H) - dists[last_node] / sqrt(2)
    probs     = softmax(10*tanh(score) + (-BIG if visited))

Sharding: batch dim across the 8 cores (pure data parallel), 8 batches per
core processed as 4 pairs of 2 batches stacked on the 128 SBUF partitions.

Host staging (layout/dtype only; all gathers, matmuls and the softmax run on
device): embeddings are shipped pre-transposed in bf16 for the score matmul
and as an fp8 copy for the visited-sum matmul; the visited mask ships as fp8
{0,1} in both layouts (with a folded ones-column so the mean rides the same
matmul); dists ship negated in fp16 fused with an emb copy so one indirect
gather per pair fetches both the distance rows and the last-node embeddings.
On device the distance bias and the -1000 mask bias are accumulated straight
into the score PSUM via identity matmuls, so the softmax chain is just
tanh -> exp(+accum) -> normalize.
"""

import json
import math
import numpy as np
from contextlib import ExitStack

import ml_dtypes

import concourse.bass as bass
import concourse.mybir as mybir
import concourse.tile as tile
from concourse.bass_utils import run_bass_kernel_spmd
from concourse.masks import make_identity


def _split_excess_waits(bir_bytes: bytes, max_waits: int = 1) -> bytes:
    """Walrus in this image rejects instructions carrying too many sem waits
    ("Too many sync wait commands", e.g. on Tile's kernel-tail Drain).
    Hoist excess waits onto preceding same-engine EventSemaphore carriers
    (pure sync ops) — sems are monotonic, so a chain of instructions whose
    waits partition the original list is equivalent."""
    d = json.loads(bir_bytes)
    n = [0]
    for fn in d.get("functions", []):
        for blk in fn.get("blocks", []):
            out = []
            for ins in blk.get("instructions", []):
                si = ins.get("sync_info") or {}
                waits = si.get("on_wait") or []
                if len(waits) > max_waits:
                    extra, keep = waits[:-max_waits], waits[-max_waits:]
                    ins["sync_info"]["on_wait"] = keep
                    for i in range(0, len(extra), max_waits):
                        n[0] += 1
                        carrier = {
                            "name": f"I-waitsplit-{n[0]}",
                            "opcode": "EventSemaphore",
                            "engine": ins["engine"],
                            "ins": [],
                            "outs": [],
                            "sync_info": {
                                "on_update": [],
                                "on_wait": extra[i:i + max_waits],
                            },
                        }
                        if "debug" in ins:
                            carrier["debug"] = ins["debug"]
                        out.append(carrier)
                out.append(ins)
            blk["instructions"] = out
    return json.dumps(d).encode()


def _install_walrus_shim():
    import concourse.bass2jax as b2j
    import concourse.bass_utils as bu
    if getattr(bu, "_waitsplit_installed", False):
        return
    real = bu.compile_bir_kernel

    def patched(bir_json, tmpdir, neff_name="file.neff", **kw):
        if isinstance(bir_json, (bytes, bytearray, str)):
            if isinstance(bir_json, str):
                bir_json = bir_json.encode()
            bir_json = _split_excess_waits(bir_json)
        return real(bir_json, tmpdir, neff_name=neff_name, **kw)

    bu.compile_bir_kernel = patched
    b2j.compile_bir_kernel = patched
    bu._waitsplit_installed = True


_install_walrus_shim()

F32 = mybir.dt.float32
F16 = mybir.dt.float16
BF = mybir.dt.bfloat16
F8 = mybir.dt.float8e4
I32 = mybir.dt.int32
OP = mybir.AluOpType
AF = mybir.ActivationFunctionType

NP_BF = ml_dtypes.bfloat16
NP_F8 = ml_dtypes.float8_e4m3

B, P, N, H = 64, 64, 1000, 128
NCORES = 8
NB = B // NCORES          # 8 batches per core
NPAIR = NB // 2           # 4 pairs
NC8 = 8                   # padded n-chunks of 128 (1024 rows, last 24 zero)

MASK_NEG = -1000.0        # additive bias for visited nodes (pre x10 exp scale)
FQ_SCALE = math.sqrt(2.0) / math.sqrt(H)   # = 0.125 exactly
TANH_SCALE = 1.0 / math.sqrt(2.0)
TANH_CLIP = 10.0


def build_nc():
    nc = bass.Bass()

    embt = nc.dram_tensor("embt", [NPAIR * H, 2 * N], BF, kind="ExternalInput")
    emb8 = nc.dram_tensor("emb8", [NPAIR * 128, 2 * NC8 * 128], F8,
                          kind="ExternalInput")
    vis8 = nc.dram_tensor("vis8", [NPAIR * 128, NC8 * 130], F8,
                          kind="ExternalInput")
    mk8 = nc.dram_tensor("mk8", [NB * P, N], F8, kind="ExternalInput")
    combo = nc.dram_tensor("combo", [NB * N, N + H], F16, kind="ExternalInput")
    idx = nc.dram_tensor("idx", [NB * P, 1], I32, kind="ExternalInput")
    eq1t = nc.dram_tensor("eq1t", [NPAIR * H, 2 * P], BF, kind="ExternalInput")
    loadv = nc.dram_tensor("loadv", [NPAIR, 128], BF, kind="ExternalInput")
    wq_graph = nc.dram_tensor("wq_graph", [H, H], BF, kind="ExternalInput")
    wq_first = nc.dram_tensor("wq_first", [H, H], BF, kind="ExternalInput")
    wq_last = nc.dram_tensor("wq_last", [H, H], BF, kind="ExternalInput")
    w_visited = nc.dram_tensor("w_visited", [H, H], BF, kind="ExternalInput")
    w_load = nc.dram_tensor("w_load", [1, H], BF, kind="ExternalInput")
    b_load = nc.dram_tensor("b_load", [1, H], BF, kind="ExternalInput")
    probs = nc.dram_tensor("probs", [NB * P, N], BF, kind="ExternalOutput")

    with tile.TileContext(nc) as tc:
        with ExitStack() as ctx:
            const = ctx.enter_context(tc.tile_pool(name="const", bufs=1))
            inp = ctx.enter_context(tc.tile_pool(name="inp", bufs=3))
            work = ctx.enter_context(tc.tile_pool(name="work", bufs=2))
            ps_sc = ctx.enter_context(
                tc.tile_pool(name="ps_sc", bufs=2, space="PSUM"))
            ps_sm = ctx.enter_context(
                tc.tile_pool(name="ps_sm", bufs=2, space="PSUM"))

            # ---- constants ----
            # fp16 identity: lastemb transpose + dist-bias pass-through matmul
            ident16 = const.tile([128, 128], F16, tag="ident16")
            make_identity(nc, ident16[:])
            # bf16 diag(-1000): mask-bias pass-through matmul (1000 exact)
            identm = const.tile([128, 128], BF, tag="identm")
            nc.gpsimd.memset(identm[:], 0.0)
            nc.gpsimd.affine_select(
                out=identm[:], in_=identm[:],
                compare_op=OP.not_equal, fill=MASK_NEG,
                base=0, pattern=[[-1, 128]], channel_multiplier=1)
            ones_row = const.tile([1, 128], BF, tag="ones_row")
            nc.gpsimd.memset(ones_row[:], 1.0)

            wg = const.tile([H, H], BF, tag="wg")
            nc.sync.dma_start(wg[:], wq_graph[:])
            wf = const.tile([H, H], BF, tag="wf")
            nc.sync.dma_start(wf[:], wq_first[:])
            wl = const.tile([H, H], BF, tag="wl")
            nc.sync.dma_start(wl[:], wq_last[:])
            wv = const.tile([H, H], BF, tag="wv")
            nc.sync.dma_start(wv[:], w_visited[:])
            wld = const.tile([1, H], BF, tag="wld")
            nc.sync.dma_start(wld[:], w_load[:])
            bld = const.tile([1, H], BF, tag="bld")
            nc.sync.dma_start(bld[:], b_load[:])

            for pr in range(NPAIR):
                r0 = 128 * pr

                # ---- loads ----
                idxr = inp.tile([128, 1], I32, tag="idxr")
                nc.sync.dma_start(idxr[:], idx[r0:r0 + 128, :])
                # one gather: rows of [-dists | emb] for last_node
                g = inp.tile([128, N + H], F16, tag="g")
                nc.gpsimd.indirect_dma_start(
                    out=g[:], out_offset=None, in_=combo[:],
                    in_offset=bass.IndirectOffsetOnAxis(ap=idxr[:, 0:1], axis=0))
                et = inp.tile([128, 2 * N], BF, tag="et")
                nc.sync.dma_start(et[:], embt[r0:r0 + 128, :])
                e8 = inp.tile([128, 2 * NC8 * 128], F8, tag="e8")
                nc.sync.dma_start(e8[:], emb8[r0:r0 + 128, :])
                v8 = inp.tile([128, NC8 * 130], F8, tag="v8")
                nc.sync.dma_start(v8[:], vis8[r0:r0 + 128, :])
                mkt = inp.tile([128, N], F8, tag="mkt")
                nc.sync.dma_start(mkt[:], mk8[r0:r0 + 128, :])
                q1 = inp.tile([128, 128], BF, tag="q1")
                nc.sync.dma_start(q1[:], eq1t[r0:r0 + 128, :])
                ldr = inp.tile([1, 128], BF, tag="ldr")
                nc.sync.dma_start(ldr[:], loadv[pr:pr + 1, :])

                # ---- q_visited-sum + mean via one fp8 matmul chain ----
                # pqv[h, 65j+q] = sum_n emb[b_j, n, h] * vis8[b_j, n, q]
                # (col 64 of each 65-block is the ones column -> N*mean)
                pqv = ps_sm.tile([128, 130], F32, tag="pqv", bufs=2)
                for c in range(NC8):
                    for j in range(2):
                        nc.tensor.matmul(
                            pqv[:, 65 * j:65 * j + 65],
                            lhsT=e8[:, 1024 * j + 128 * c:1024 * j + 128 * c + 128],
                            rhs=v8[:, 130 * c + 65 * j:130 * c + 65 * j + 65],
                            start=(c == 0), stop=(c == NC8 - 1))
                qvs = work.tile([128, 128], BF, tag="qvs")
                nc.vector.tensor_scalar(out=qvs[:, 0:64], in0=pqv[:, 0:64],
                                        scalar1=1.0 / N, scalar2=None,
                                        op0=OP.mult)
                nc.vector.tensor_scalar(out=qvs[:, 64:128], in0=pqv[:, 65:129],
                                        scalar1=1.0 / N, scalar2=None,
                                        op0=OP.mult)
                ms = work.tile([128, 2], F32, tag="ms")
                nc.vector.tensor_copy(out=ms[:, 0:1], in_=pqv[:, 64:65])
                nc.vector.tensor_copy(out=ms[:, 1:2], in_=pqv[:, 129:130])
                mrep = work.tile([128, 128], BF, tag="mrep")
                for j in range(2):
                    nc.vector.tensor_scalar(
                        out=mrep[:, 64 * j:64 * j + 64],
                        in0=ms[:, j:j + 1].to_broadcast([128, 64]),
                        scalar1=1.0 / N, scalar2=None, op0=OP.mult)

                # ---- last-node embedding, transposed to [h, 2p] ----
                ptr = ps_sm.tile([128, 128], F16, tag="ptr", bufs=1)
                nc.tensor.transpose(out=ptr[:], in_=g[:, N:N + H],
                                    identity=ident16[:])
                lastT = work.tile([128, 128], BF, tag="lastT")
                nc.vector.tensor_copy(out=lastT[:], in_=ptr[:])

                # ---- final_q^T accumulation: psum [h, 2p] ----
                pfq = ps_sm.tile([128, 128], F32, tag="pfq", bufs=1)
                nc.tensor.matmul(pfq[:], lhsT=wf[:], rhs=q1[:],
                                 start=True, stop=False)
                nc.tensor.matmul(pfq[:], lhsT=wl[:], rhs=lastT[:],
                                 start=False, stop=False)
                nc.tensor.matmul(pfq[:], lhsT=wg[:], rhs=mrep[:],
                                 start=False, stop=False)
                nc.tensor.matmul(pfq[:], lhsT=wv[:], rhs=qvs[:],
                                 start=False, stop=False)
                nc.tensor.matmul(pfq[:], lhsT=wld[:], rhs=ldr[:],
                                 start=False, stop=False)
                nc.tensor.matmul(pfq[:], lhsT=bld[:], rhs=ones_row[:],
                                 start=False, stop=True)
                fqT = work.tile([128, 128], BF, tag="fqT")
                nc.vector.tensor_scalar(out=fqT[:], in0=pfq[:],
                                        scalar1=FQ_SCALE, scalar2=None,
                                        op0=OP.mult)

                # ---- score psum = q.e*0.125 - dist - 1000*mask; then tanh ----
                t = work.tile([128, N], F32, tag="t")
                for (n0, n1) in ((0, 512), (512, N)):
                    psc = ps_sc.tile([128, n1 - n0], F32, tag=f"psc{n0}")
                    # dist bias: combo rows hold -dists, identity passes through
                    nc.tensor.matmul(psc[:], lhsT=ident16[:], rhs=g[:, n0:n1],
                                     start=True, stop=False)
                    # mask bias: diag(-1000) x {0,1}
                    nc.tensor.matmul(psc[:], lhsT=identm[:], rhs=mkt[:, n0:n1],
                                     start=False, stop=False)
                    for j in range(2):
                        nc.tensor.matmul(
                            psc[64 * j:64 * j + 64, :],
                            lhsT=fqT[:, 64 * j:64 * j + 64],
                            rhs=et[:, N * j + n0:N * j + n1],
                            start=False, stop=(j == 1))
                    nc.scalar.activation(t[:, n0:n1], psc[:], AF.Tanh,
                                         scale=TANH_SCALE)

                # ---- softmax ----
                e = work.tile([128, N], F32, tag="e")
                ssum = work.tile([128, 1], F32, tag="ssum")
                nc.scalar.activation(e[:], t[:], AF.Exp, scale=TANH_CLIP,
                                     accum_out=ssum[:])
                rec = work.tile([128, 1], F32, tag="rec")
                nc.vector.reciprocal(out=rec[:], in_=ssum[:])
                pout = work.tile([128, N], BF, tag="pout")
                nc.vector.tensor_tensor(
                    out=pout[:], in0=e[:],
                    in1=rec[:, 0:1].to_broadcast([128, N]), op=OP.mult)
                nc.sync.dma_start(probs[r0:r0 + 128, :], pout[:])

    return nc


_CACHE = {}


def _get_nc():
    if "nc" not in _CACHE:
        _CACHE["nc"] = build_nc()
    return _CACHE["nc"]


def _shard_inputs(inputs):
    dists = np.asarray(inputs["dists"], dtype=np.float32)        # [B,N,N]
    emb = np.asarray(inputs["embeddings"], dtype=np.float32)     # [B,N,H]
    eq1 = np.asarray(inputs["encoded_q1"], dtype=np.float32)     # [B,P,H]
    last = np.asarray(inputs["last_node"]).astype(np.int64)      # [B,P]
    load = np.asarray(inputs["load"], dtype=np.float32)          # [B,P]
    mask = np.asarray(inputs["group_ninf_mask"], dtype=np.float32)
    vis = np.isneginf(mask)                                      # bool [B,P,N]

    # fused gather table: row (b,n) = [-dists[b,n,:] | emb[b,n,:]] in fp16
    combo = np.empty((B, N, N + H), dtype=np.float16)
    combo[:, :, :N] = -dists
    combo[:, :, N:] = emb

    # embT pair-packed bf16: [pair, h, j*N+n]
    embT = np.ascontiguousarray(emb.transpose(0, 2, 1))          # [B,H,N]

    # emb as fp8, n-chunked with n padded to 1024: [B, 128, chunk, h]
    embp = np.zeros((B, NC8 * 128, H), dtype=np.float32)
    embp[:, :N] = emb
    emb8 = np.ascontiguousarray(
        embp.reshape(B, NC8, 128, H).transpose(0, 2, 1, 3)).astype(NP_F8)

    # visited^T with ones column (for mean): [B, 128, chunk, 65]
    v = np.zeros((B, NC8 * 128, P + 1), dtype=np.float32)
    v[:, :N, :P] = vis.transpose(0, 2, 1)
    v[:, :, P] = 1.0
    vis8 = np.ascontiguousarray(
        v.reshape(B, NC8, 128, P + 1).transpose(0, 2, 1, 3)).astype(NP_F8)

    mk8 = vis.astype(NP_F8)                                      # [B,P,N] {0,1}
    eq1T = np.ascontiguousarray(eq1.transpose(0, 2, 1))          # [B,H,P]
    # flat row index into the per-core [NB*N, :] gather table
    idxflat = (last + (np.arange(B) % NB)[:, None] * N).astype(np.int32)

    w_bf = {k: np.asarray(inputs[k], dtype=np.float32).astype(NP_BF)
            for k in ("Wq_graph", "Wq_first", "Wq_last", "W_visited")}
    wld = np.asarray(inputs["W_load"], dtype=np.float32).reshape(1, H).astype(NP_BF)
    bld = np.asarray(inputs["b_load"], dtype=np.float32).reshape(1, H).astype(NP_BF)

    in_maps = []
    for c in range(NCORES):
        s = slice(c * NB, (c + 1) * NB)
        embT_c = embT[s].reshape(NPAIR, 2, H, N).transpose(0, 2, 1, 3) \
            .reshape(NPAIR * H, 2 * N).astype(NP_BF)
        emb8_c = emb8[s].reshape(NPAIR, 2, 128, NC8, 128) \
            .transpose(0, 2, 1, 3, 4).reshape(NPAIR * 128, 2 * NC8 * 128)
        vis8_c = vis8[s].reshape(NPAIR, 2, 128, NC8, P + 1) \
            .transpose(0, 2, 3, 1, 4).reshape(NPAIR * 128, NC8 * 130)
        eq1T_c = eq1T[s].reshape(NPAIR, 2, H, P).transpose(0, 2, 1, 3) \
            .reshape(NPAIR * H, 2 * P).astype(NP_BF)
        in_maps.append(dict(
            embt=np.ascontiguousarray(embT_c),
            emb8=np.ascontiguousarray(emb8_c),
            vis8=np.ascontiguousarray(vis8_c),
            mk8=mk8[s].reshape(NB * P, N),
            combo=combo[s].reshape(NB * N, N + H),
            idx=idxflat[s].reshape(NB * P, 1),
            eq1t=np.ascontiguousarray(eq1T_c),
            loadv=load[s].reshape(NPAIR, 128).astype(NP_BF),
            wq_graph=w_bf["Wq_graph"],
            wq_first=w_bf["Wq_first"],
            wq_last=w_bf["Wq_last"],
            w_visited=w_bf["W_visited"],
            w_load=wld,
            b_load=bld,
        ))
    return in_maps


def _run(inputs, trace=False, **kw):
    nc = _get_nc()
    in_maps = _shard_inputs(inputs)
    res = run_bass_kernel_spmd(nc, in_maps, list(range(NCORES)),
                               trace=trace, **kw)
    out = np.concatenate(
        [np.asarray(r["probs"]).astype(np.float32).reshape(NB, P, N)
         for r in res.results], axis=0)
    return out, res


def kernel(**inputs) -> np.ndarray:
    out, _ = _run(inputs)
    return out


# revision 15
# speedup vs baseline: 1.7455x; 1.0005x over previous
"""CVRP decoder kernel for Trainium2 (8 NeuronCores, batch-data-parallel).

Computes, per batch b (B=64, P=64, N=1000, H=128):
    q_graph   = mean_n(emb) @ Wq_graph
    q_first   = encoded_q1 @ Wq_first
    q_last    = emb[last_node] @ Wq_last
    q_visited = (vis01 @ emb / N) @ W_visited          (vis01 = isneginf(mask))
    final_q   = sum of the above + load*W_load + b_load
    score     = final_q @ emb^T / sqrt(H) - dists[last_node] / sqrt(2)
    probs     = softmax(10*tanh(score) + (-BIG if visited))

Sharding: batch dim across the 8 cores (pure data parallel), 8 batches per
core processed as 4 pairs of 2 batches stacked on the 128 SBUF partitions.

Host staging (layout/dtype only; all gathers, matmuls and the softmax run on
device): per pair, all dense inputs are byte-packed into ONE mega row
(embT bf16 | emb fp8 chunked | visited^T fp8 (+ones col for the mean) |
visited fp8 | eq1T bf16) so a single DMA per pair loads everything; dists
ship negated in fp16 fused with an emb copy so one indirect gather per pair
fetches both the distance rows and last-node embeddings.  On device the
distance bias and the -1000 mask bias are accumulated straight into the
score PSUM via identity matmuls, so the softmax chain is just
tanh -> exp(+accum) -> normalize.
"""

import json
import math
import numpy as np
from contextlib import ExitStack

import ml_dtypes

import concourse.bass as bass
import concourse.mybir as mybir
import concourse.tile as tile
from concourse.bass_utils import run_bass_kernel_spmd
from concourse.masks import make_identity


def _split_excess_waits(bir_bytes: bytes, max_waits: int = 1) -> bytes:
    """Walrus in this image rejects instructions carrying too many sem waits
    ("Too many sync wait commands", e.g. on Tile's kernel-tail Drain).
    Hoist excess waits onto preceding same-engine EventSemaphore carriers
    (pure sync ops) — sems are monotonic, so a chain of instructions whose
    waits partition the original list is equivalent."""
    d = json.loads(bir_bytes)
    n = [0]
    for fn in d.get("functions", []):
        for blk in fn.get("blocks", []):
            out = []
            for ins in blk.get("instructions", []):
                si = ins.get("sync_info") or {}
                waits = si.get("on_wait") or []
                if len(waits) > max_waits:
                    extra, keep = waits[:-max_waits], waits[-max_waits:]
                    ins["sync_info"]["on_wait"] = keep
                    for i in range(0, len(extra), max_waits):
                        n[0] += 1
                        carrier = {
                            "name": f"I-waitsplit-{n[0]}",
                            "opcode": "EventSemaphore",
                            "engine": ins["engine"],
                            "ins": [],
                            "outs": [],
                            "sync_info": {
                                "on_update": [],
                                "on_wait": extra[i:i + max_waits],
                            },
                        }
                        if "debug" in ins:
                            carrier["debug"] = ins["debug"]
                        out.append(carrier)
                out.append(ins)
            blk["instructions"] = out
    return json.dumps(d).encode()


def _install_walrus_shim():
    import concourse.bass2jax as b2j
    import concourse.bass_utils as bu
    if getattr(bu, "_waitsplit_installed", False):
        return
    real = bu.compile_bir_kernel

    def patched(bir_json, tmpdir, neff_name="file.neff", **kw):
        if isinstance(bir_json, (bytes, bytearray, str)):
            if isinstance(bir_json, str):
                bir_json = bir_json.encode()
            bir_json = _split_excess_waits(bir_json)
        return real(bir_json, tmpdir, neff_name=neff_name, **kw)

    bu.compile_bir_kernel = patched
    b2j.compile_bir_kernel = patched
    bu._waitsplit_installed = True


_install_walrus_shim()

F32 = mybir.dt.float32
F16 = mybir.dt.float16
BF = mybir.dt.bfloat16
F8 = mybir.dt.float8e4
U8 = mybir.dt.uint8
I32 = mybir.dt.int32
OP = mybir.AluOpType
AF = mybir.ActivationFunctionType

NP_BF = ml_dtypes.bfloat16
NP_F8 = ml_dtypes.float8_e4m3

B, P, N, H = 64, 64, 1000, 128
NCORES = 8
NB = B // NCORES          # 8 batches per core
NPAIR = NB // 2           # 4 pairs
NC8 = 8                   # padded n-chunks of 128 (1024 rows, last 24 zero)

MASK_NEG = -1000.0        # additive bias for visited nodes (pre x10 exp scale)
FQ_SCALE = math.sqrt(2.0) / math.sqrt(H)   # = 0.125 exactly
TANH_SCALE = 1.0 / math.sqrt(2.0)
TANH_CLIP = 10.0

# mega input row byte layout (per pair, per partition row)
MEG_ET = 0                # embT bf16 [2000]  (j*1000+n)
MEG_E8 = 4000             # emb fp8 [2*8*128] (j,chunk,h)
MEG_V8 = 6048             # visited^T+ones fp8 [8*130] (chunk, j*65+q)
MEG_MK = 7088             # visited fp8 [1000] (row-major [p,n])
MEG_Q1 = 8088             # eq1T bf16 [128]   (j*64+p)
MEG_BYTES = 8344


def build_nc():
    nc = bass.Bass()

    mega = nc.dram_tensor("mega", [NPAIR * 128, MEG_BYTES], U8,
                          kind="ExternalInput")
    combo = nc.dram_tensor("combo", [NB * N, N + H], F16, kind="ExternalInput")
    idxt = nc.dram_tensor("idxt", [P * 2, NPAIR], I32, kind="ExternalInput")
    ldt = nc.dram_tensor("ldt", [1, NPAIR * 128], BF, kind="ExternalInput")
    wqt = nc.dram_tensor("wqt", [H, 4 * H], BF, kind="ExternalInput")
    wldt = nc.dram_tensor("wldt", [1, 2 * H], BF, kind="ExternalInput")
    probs = nc.dram_tensor("probs", [NB * P, N], BF, kind="ExternalOutput")

    with tile.TileContext(nc) as tc:
        with ExitStack() as ctx:
            const = ctx.enter_context(tc.tile_pool(name="const", bufs=1))
            inp = ctx.enter_context(tc.tile_pool(name="inp", bufs=4))
            work = ctx.enter_context(tc.tile_pool(name="work", bufs=2))
            ps_sc = ctx.enter_context(
                tc.tile_pool(name="ps_sc", bufs=2, space="PSUM"))
            ps_sm = ctx.enter_context(
                tc.tile_pool(name="ps_sm", bufs=2, space="PSUM"))

            # ---- constants ----
            # fp16 identity: lastemb transpose + dist-bias pass-through matmul
            ident16 = const.tile([128, 128], F16, tag="ident16")
            make_identity(nc, ident16[:])
            # bf16 diag(-1000): mask-bias pass-through matmul (1000 exact)
            identm = const.tile([128, 128], BF, tag="identm")
            nc.gpsimd.memset(identm[:], 0.0)
            nc.gpsimd.affine_select(
                out=identm[:], in_=identm[:],
                compare_op=OP.not_equal, fill=MASK_NEG,
                base=0, pattern=[[-1, 128]], channel_multiplier=1)
            ones_row = const.tile([1, 128], BF, tag="ones_row")
            nc.gpsimd.memset(ones_row[:], 1.0)

            wq = const.tile([H, 4 * H], BF, tag="wq")
            nc.sync.dma_start(wq[:], wqt[:])
            wlb = const.tile([1, 2 * H], BF, tag="wlb")
            nc.sync.dma_start(wlb[:], wldt[:])
            idxa = const.tile([P * 2, NPAIR], I32, tag="idxa")
            nc.sync.dma_start(idxa[:], idxt[:])
            lda = const.tile([1, NPAIR * 128], BF, tag="lda")
            nc.sync.dma_start(lda[:], ldt[:])
            wg, wf, wl, wv = (wq[:, 128 * k:128 * (k + 1)] for k in range(4))

            # ---- issue all input loads up front (bufs=4 -> no WAR waits) ----
            megs, gs = [], []
            for pr in range(NPAIR):
                r0 = 128 * pr
                m = inp.tile([128, MEG_BYTES], U8, tag="mega")
                nc.sync.dma_start(m[:], mega[r0:r0 + 128, :])
                megs.append(m)
                g = inp.tile([128, N + H], F16, tag="g")
                nc.gpsimd.indirect_dma_start(
                    out=g[:], out_offset=None, in_=combo[:],
                    in_offset=bass.IndirectOffsetOnAxis(
                        ap=idxa[:, pr:pr + 1], axis=0))
                gs.append(g)

            for pr in range(NPAIR):
                r0 = 128 * pr
                m, g = megs[pr], gs[pr]
                et = m[:, MEG_ET:MEG_E8].bitcast(BF)       # [128, 2000]
                e8 = m[:, MEG_E8:MEG_V8].bitcast(F8)       # [128, 2048]
                v8 = m[:, MEG_V8:MEG_MK].bitcast(F8)       # [128, 1040]
                mkt = m[:, MEG_MK:MEG_Q1].bitcast(F8)      # [128, 1000]
                q1 = m[:, MEG_Q1:MEG_BYTES].bitcast(BF)    # [128, 128]

                # ---- visited-sum + mean via one fp8 matmul chain ----
                # pqv[h, 65j+q] = sum_n emb[b_j, n, h] * vis8[b_j, n, q]
                # (col 64 of each 65-block is the ones column -> N*mean)
                pqv = ps_sm.tile([128, 130], F32, tag="pqv", bufs=2)
                for c in range(NC8):
                    for j in range(2):
                        nc.tensor.matmul(
                            pqv[:, 65 * j:65 * j + 65],
                            lhsT=e8[:, 1024 * j + 128 * c:1024 * j + 128 * c + 128],
                            rhs=v8[:, 130 * c + 65 * j:130 * c + 65 * j + 65],
                            start=(c == 0), stop=(c == NC8 - 1))
                qvs = work.tile([128, 128], BF, tag="qvs")
                nc.vector.tensor_scalar(out=qvs[:, 0:64], in0=pqv[:, 0:64],
                                        scalar1=1.0 / N, scalar2=None,
                                        op0=OP.mult)
                nc.vector.tensor_scalar(out=qvs[:, 64:128], in0=pqv[:, 65:129],
                                        scalar1=1.0 / N, scalar2=None,
                                        op0=OP.mult)
                mrep = work.tile([128, 128], BF, tag="mrep")
                for j in range(2):
                    nc.vector.tensor_scalar(
                        out=mrep[:, 64 * j:64 * j + 64],
                        in0=pqv[:, 65 * j + 64:65 * j + 65].to_broadcast([128, 64]),
                        scalar1=1.0 / N, scalar2=None, op0=OP.mult)

                # ---- last-node embedding, transposed to [h, 2p] ----
                ptr = ps_sm.tile([128, 128], F16, tag="ptr", bufs=1)
                nc.tensor.transpose(out=ptr[:], in_=g[:, N:N + H],
                                    identity=ident16[:])
                lastT = work.tile([128, 128], BF, tag="lastT")
                nc.vector.tensor_copy(out=lastT[:], in_=ptr[:])

                # ---- final_q^T accumulation: psum [h, 2p] ----
                pfq = ps_sm.tile([128, 128], F32, tag="pfq", bufs=1)
                nc.tensor.matmul(pfq[:], lhsT=wf, rhs=q1[:, :],
                                 start=True, stop=False)
                nc.tensor.matmul(pfq[:], lhsT=wl, rhs=lastT[:],
                                 start=False, stop=False)
                nc.tensor.matmul(pfq[:], lhsT=wg, rhs=mrep[:],
                                 start=False, stop=False)
                nc.tensor.matmul(pfq[:], lhsT=wv, rhs=qvs[:],
                                 start=False, stop=False)
                nc.tensor.matmul(pfq[:], lhsT=wlb[0:1, 0:H],
                                 rhs=lda[0:1, 128 * pr:128 * pr + 128],
                                 start=False, stop=False)
                nc.tensor.matmul(pfq[:], lhsT=wlb[0:1, H:2 * H],
                                 rhs=ones_row[:],
                                 start=False, stop=True)
                fqT = work.tile([128, 128], BF, tag="fqT")
                nc.vector.tensor_scalar(out=fqT[:], in0=pfq[:],
                                        scalar1=FQ_SCALE, scalar2=None,
                                        op0=OP.mult)

                # ---- score psum = q.e*0.125 - dist - 1000*mask ----
                psc = ps_sc.tile([128, N], F32, tag="psc")
                for (n0, n1) in ((0, 512), (512, N)):
                    # dist bias: combo rows hold -dists, identity passes through
                    nc.tensor.matmul(psc[:, n0:n1], lhsT=ident16[:],
                                     rhs=g[:, n0:n1], start=True, stop=False)
                    # mask bias: diag(-1000) x {0,1}
                    nc.tensor.matmul(psc[:, n0:n1], lhsT=identm[:],
                                     rhs=mkt[:, n0:n1], start=False, stop=False)
                    for j in range(2):
                        nc.tensor.matmul(
                            psc[64 * j:64 * j + 64, n0:n1],
                            lhsT=fqT[:, 64 * j:64 * j + 64],
                            rhs=et[:, N * j + n0:N * j + n1],
                            start=False, stop=(j == 1))

                # ---- tanh / softmax ----
                t = work.tile([128, N], F32, tag="t")
                nc.scalar.activation(t[:], psc[:], AF.Tanh, scale=TANH_SCALE)
                e = work.tile([128, N], F32, tag="e")
                ssum = work.tile([128, 1], F32, tag="ssum")
                nc.scalar.activation(e[:], t[:], AF.Exp, scale=TANH_CLIP,
                                     accum_out=ssum[:])
                rec = work.tile([128, 1], F32, tag="rec")
                nc.vector.reciprocal(out=rec[:], in_=ssum[:])
                pout = work.tile([128, N], BF, tag="pout")
                nc.vector.tensor_tensor(
                    out=pout[:], in0=e[:],
                    in1=rec[:, 0:1].to_broadcast([128, N]), op=OP.mult)
                nc.sync.dma_start(probs[r0:r0 + 128, :], pout[:])

    return nc


_CACHE = {}


def _get_nc():
    if "nc" not in _CACHE:
        _CACHE["nc"] = build_nc()
    return _CACHE["nc"]


def _shard_inputs(inputs):
    dists = np.asarray(inputs["dists"], dtype=np.float32)        # [B,N,N]
    emb = np.asarray(inputs["embeddings"], dtype=np.float32)     # [B,N,H]
    eq1 = np.asarray(inputs["encoded_q1"], dtype=np.float32)     # [B,P,H]
    last = np.asarray(inputs["last_node"]).astype(np.int64)      # [B,P]
    load = np.asarray(inputs["load"], dtype=np.float32)          # [B,P]
    mask = np.asarray(inputs["group_ninf_mask"], dtype=np.float32)
    vis = np.isneginf(mask)                                      # bool [B,P,N]

    # fused gather table: row (b,n) = [-dists[b,n,:] | emb[b,n,:]] in fp16
    combo = np.empty((B, N, N + H), dtype=np.float16)
    combo[:, :, :N] = -dists
    combo[:, :, N:] = emb

    # embT pair-packed bf16 view source
    embT = np.ascontiguousarray(emb.transpose(0, 2, 1))          # [B,H,N]

    # emb as fp8, n-chunked with n padded to 1024: [B, 128, chunk, h]
    embp = np.zeros((B, NC8 * 128, H), dtype=np.float32)
    embp[:, :N] = emb
    emb8 = np.ascontiguousarray(
        embp.reshape(B, NC8, 128, H).transpose(0, 2, 1, 3)).astype(NP_F8)

    # visited^T with ones column (for mean): [B, 128, chunk, 65]
    v = np.zeros((B, NC8 * 128, P + 1), dtype=np.float32)
    v[:, :N, :P] = vis.transpose(0, 2, 1)
    v[:, :, P] = 1.0
    vis8 = np.ascontiguousarray(
        v.reshape(B, NC8, 128, P + 1).transpose(0, 2, 1, 3)).astype(NP_F8)

    mk8 = vis.astype(NP_F8)                                      # [B,P,N] {0,1}
    eq1T = np.ascontiguousarray(eq1.transpose(0, 2, 1))          # [B,H,P]
    # flat row index into the per-core [NB*N, :] gather table
    idxflat = (last + (np.arange(B) % NB)[:, None] * N).astype(np.int32)

    # packed order must match wg, wf, wl, wv slices
    wq_pack = np.concatenate(
        [np.asarray(inputs["Wq_graph"], dtype=np.float32),
         np.asarray(inputs["Wq_first"], dtype=np.float32),
         np.asarray(inputs["Wq_last"], dtype=np.float32),
         np.asarray(inputs["W_visited"], dtype=np.float32)], axis=1)
    wq_pack = wq_pack.astype(NP_BF)
    wlb = np.concatenate(
        [np.asarray(inputs["W_load"], dtype=np.float32),
         np.asarray(inputs["b_load"], dtype=np.float32)]
    ).reshape(1, 2 * H).astype(NP_BF)                            # [1, 2H]

    in_maps = []
    u8 = np.uint8
    for c in range(NCORES):
        s = slice(c * NB, (c + 1) * NB)
        embT_c = embT[s].reshape(NPAIR, 2, H, N).transpose(0, 2, 1, 3) \
            .reshape(NPAIR * H, 2 * N).astype(NP_BF)
        emb8_c = emb8[s].reshape(NPAIR, 2, 128, NC8, 128) \
            .transpose(0, 2, 1, 3, 4).reshape(NPAIR * 128, 2 * NC8 * 128)
        vis8_c = vis8[s].reshape(NPAIR, 2, 128, NC8, P + 1) \
            .transpose(0, 2, 3, 1, 4).reshape(NPAIR * 128, NC8 * 130)
        eq1T_c = eq1T[s].reshape(NPAIR, 2, H, P).transpose(0, 2, 1, 3) \
            .reshape(NPAIR * H, 2 * P).astype(NP_BF)
        meg = np.empty((NPAIR * 128, MEG_BYTES), dtype=u8)
        meg[:, MEG_ET:MEG_E8] = np.ascontiguousarray(embT_c).view(u8)
        meg[:, MEG_E8:MEG_V8] = np.ascontiguousarray(emb8_c).view(u8)
        meg[:, MEG_V8:MEG_MK] = np.ascontiguousarray(vis8_c).view(u8)
        meg[:, MEG_MK:MEG_Q1] = mk8[s].reshape(NPAIR * 128, N).view(u8)
        meg[:, MEG_Q1:MEG_BYTES] = np.ascontiguousarray(eq1T_c).view(u8)
        in_maps.append(dict(
            mega=meg,
            combo=combo[s].reshape(NB * N, N + H),
            idxt=np.ascontiguousarray(
                idxflat[s].reshape(NPAIR, 2 * P).T),             # [128, NPAIR]
            ldt=load[s].reshape(1, NPAIR * 128).astype(NP_BF),
            wqt=wq_pack,
            wldt=wlb,
        ))
    return in_maps


def _run(inputs, trace=False, **kw):
    nc = _get_nc()
    in_maps = _shard_inputs(inputs)
    res = run_bass_kernel_spmd(nc, in_maps, list(range(NCORES)),
                               trace=trace, **kw)
    out = np.concatenate(
        [np.asarray(r["probs"]).astype(np.float32).reshape(NB, P, N)
         for r in res.results], axis=0)
    return out, res


def kernel(**inputs) -> np.ndarray:
    out, _ = _run(inputs)
    return out
